# revision 7
# baseline (speedup 1.0000x reference)
"""Differentiable FE solver (2D P1 FEM Poisson, 64x64 structured grid) on TRN2.

Pipeline (all floating-point work on device, replicated SPMD on 8 cores):
  1. Element assembly: per-element geometry (b, c, area), local stiffness
     Ke = kappa*(b_p b_q + c_p c_q)/(4 area) and load fe = area/3 * mean(f).
     The mesh topology (from the int32 `elements` input) is cell-regular, so
     every gather/scatter becomes a shifted 2D-slice add on 64x64 node planes
     -- no indexed DMA needed.  The assembled operator is kept in stencil form
     (7 direction planes) instead of a dense 4096^2 K.
  2. Dirichlet elimination: F0 = F - K*u_bc (stencil matvec); boundary rows
     are dropped by only ever reading interior slices.
  3. Solve K_free u = F0 by DST-preconditioned iterative refinement: the exact
     inverse of the constant-coefficient Laplacian on the grid is
     S diag(1/(lam_i+lam_j)) S (S = 62x62 sine matrix), applied as 4 small
     matmuls on the PE.  One refinement step against the *assembled* K (so the
     answer tracks the actual inputs, not the idealized operator) reaches
     ~1e-6 relative error.

Engine access patterns may only start at partitions 0/32/64/96, so all
partition-dimension shifts (the "row" index of the grid) are realigned with
small SBUF->SBUF DMAs; free-dimension shifts are plain AP offsets.

Host side only derives integer layout plans from the int32 topology inputs,
reshapes/permutes arrays, and emits constant tables (sine matrix, eigenvalue
plane); every float computation happens in the Bass kernel.
"""

import numpy as np

import concourse.bacc as bacc
import concourse.mybir as mybir
import concourse.tile as tile
from concourse.bass_utils import run_bass_kernel_spmd

N = 64            # nodes per side
M = N - 1         # cells per side
NI = N - 2        # interior nodes per side
NCORES = 8
AREA_EPS = 1e-15

_CACHE = {}


def _host_plan(elements, free_idx, dir_idx):
    """Derive the cell-regular layout plan from int32 topology inputs."""
    el = elements.astype(np.int64)
    ga, gb = el // N, el % N
    ne = el.shape[0]
    assert ne == 2 * M * M, ne
    ncell = ne // 2
    ca, cb = np.meshgrid(np.arange(M), np.arange(M), indexing="ij")
    cells = np.stack([ca.ravel(), cb.ravel()], 1)
    offs = np.zeros((2, 3, 2), np.int64)
    for tau in (0, 1):
        es = slice(tau * ncell, (tau + 1) * ncell)
        for p in range(3):
            d = np.stack([ga[es, p], gb[es, p]], 1) - cells
            assert (d == d[0]).all(), "mesh is not cell-regular"
            assert d[0, 0] in (0, 1) and d[0, 1] in (0, 1)
            offs[tau, p] = d[0]
    idx = np.arange(N * N).reshape(N, N)
    bmask = np.zeros(N * N, bool)
    bmask[idx[0, :]] = True
    bmask[idx[-1, :]] = True
    bmask[idx[:, 0]] = True
    bmask[idx[:, -1]] = True
    assert (free_idx == np.nonzero(~bmask)[0]).all(), "free_idx mismatch"
    assert (dir_idx == np.nonzero(bmask)[0]).all(), "dir_idx mismatch"
    return offs


def _build_program(offs):
    f32 = mybir.dt.float32
    nc = bacc.Bacc("TRN2", target_bir_lowering=False, debug=False,
                   num_devices=NCORES)

    d_X = nc.dram_tensor("X", [N, N], f32, kind="ExternalInput")
    d_Y = nc.dram_tensor("Y", [N, N], f32, kind="ExternalInput")
    d_FG = nc.dram_tensor("FG", [N, N], f32, kind="ExternalInput")
    d_UBC = nc.dram_tensor("UBC", [N, N], f32, kind="ExternalInput")
    d_S = nc.dram_tensor("S", [NI, NI], f32, kind="ExternalInput")
    d_IL = nc.dram_tensor("IL", [NI, NI], f32, kind="ExternalInput")
    d_KAP = nc.dram_tensor("KAP", [1, 1], f32, kind="ExternalInput")
    d_U = nc.dram_tensor("U", [N, N], f32, kind="ExternalOutput")

    # coupling directions present in this mesh, and which of them receive
    # contributions from vertices with cell-row offset 1 (these need a
    # partition realignment before the final combine)
    dirs = []
    needs_shift = set()
    for tau in (0, 1):
        for p in range(3):
            for q in range(3):
                d = (int(offs[tau, q, 0] - offs[tau, p, 0]),
                     int(offs[tau, q, 1] - offs[tau, p, 1]))
                if d not in dirs:
                    dirs.append(d)
                if offs[tau, p, 0] == 1:
                    needs_shift.add(d)

    with tile.TileContext(nc) as tc:
        with (
            tc.tile_pool(name="io", bufs=1) as io,
            tc.tile_pool(name="wk", bufs=1) as wk,
            tc.tile_pool(name="ps", bufs=2, space="PSUM") as ps,
        ):
            X = io.tile([N, N], f32, tag="X")
            Y = io.tile([N, N], f32, tag="Y")
            FG = io.tile([N, N], f32, tag="FG")
            UBC = io.tile([N, N], f32, tag="UBC")
            S = io.tile([NI, NI], f32, tag="S")
            IL = io.tile([NI, NI], f32, tag="IL")
            KAP = io.tile([1, 1], f32, tag="KAP")
            nc.gpsimd.dma_start(X[:], d_X[:])
            nc.gpsimd.dma_start(Y[:], d_Y[:])
            nc.gpsimd.dma_start(FG[:], d_FG[:])
            nc.gpsimd.dma_start(UBC[:], d_UBC[:])
            nc.gpsimd.dma_start(S[:], d_S[:])
            nc.gpsimd.dma_start(IL[:], d_IL[:])
            nc.gpsimd.dma_start(KAP[:], d_KAP[:])

            # row-shifted input planes: XS[a] = X[a+1] etc. (row 63 unused)
            XS = wk.tile([N, N], f32, tag="XS")
            YS = wk.tile([N, N], f32, tag="YS")
            FS = wk.tile([N, N], f32, tag="FS")
            nc.gpsimd.dma_start(XS[0:M, :], X[1:N, :])
            nc.gpsimd.dma_start(YS[0:M, :], Y[1:N, :])
            nc.gpsimd.dma_start(FS[0:M, :], FG[1:N, :])

            # broadcast kappa and 1/kappa down the partition dim via the PE
            kinv = wk.tile([1, 1], f32, tag="kinv")
            nc.vector.reciprocal(kinv[:], KAP[:])
            ones = wk.tile([1, M], f32, tag="ones")
            nc.gpsimd.memset(ones[:], 1.0)
            kap_ps = ps.tile([M, 1], f32, tag="kbc")
            nc.tensor.matmul(kap_ps[:], ones[:], KAP[:], start=True, stop=True)
            kap_b = wk.tile([M, 1], f32, tag="kap_b")
            nc.vector.tensor_copy(kap_b[:], kap_ps[:])
            kinv_ps = ps.tile([M, 1], f32, tag="kbc")
            nc.tensor.matmul(kinv_ps[:], ones[:], kinv[:], start=True, stop=True)
            kinv_b = wk.tile([M, 1], f32, tag="kinv_b")
            nc.vector.tensor_copy(kinv_b[:], kinv_ps[:])
            # ILK = IL / kappa
            ILK = wk.tile([NI, NI], f32, tag="ILK")
            nc.vector.tensor_scalar(ILK[:], IL[:], kinv_b[0:NI, 0:1], None,
                                    op0=mybir.AluOpType.mult)

            # per-direction accumulators: W0 for cell-row-0 vertices (already
            # node-aligned), W1 for cell-row-1 vertices (node row = cell row+1,
            # realigned by DMA at the end)
            W0, W1 = {}, {}
            for d in dirs:
                W0[d] = wk.tile([N, N], f32, tag=f"W0_{d}", name=f"W0_{d}")
                nc.vector.memzero(W0[d][:])
                if d in needs_shift:
                    W1[d] = wk.tile([N, N], f32, tag=f"W1_{d}", name=f"W1_{d}")
                    nc.vector.memzero(W1[d][:])
            F0acc = wk.tile([N, N], f32, tag="F0acc")
            F1acc = wk.tile([N, N], f32, tag="F1acc")
            nc.vector.memzero(F0acc[:])
            nc.vector.memzero(F1acc[:])

            def src(plane, shifted, tau, p):
                """cell plane of vertex-p values: [cell_a 0..62, cell_b 0..62]."""
                oa, ob = int(offs[tau, p, 0]), int(offs[tau, p, 1])
                t = shifted if oa == 1 else plane
                return t[0:M, ob:ob + M]

            for tau in (0, 1):
                xs = [src(X, XS, tau, p) for p in range(3)]
                ys = [src(Y, YS, tau, p) for p in range(3)]
                fs = [src(FG, FS, tau, p) for p in range(3)]

                def mk(tag):
                    return wk.tile([M, M], f32, tag=f"{tag}{tau}",
                                   name=f"{tag}{tau}")

                t1 = mk("t1"); nc.vector.tensor_sub(t1[:], xs[1], xs[0])
                t2 = mk("t2"); nc.vector.tensor_sub(t2[:], ys[2], ys[0])
                t3 = mk("t3"); nc.vector.tensor_sub(t3[:], xs[2], xs[0])
                t4 = mk("t4"); nc.vector.tensor_sub(t4[:], ys[1], ys[0])
                d1 = mk("d1"); nc.vector.tensor_mul(d1[:], t1[:], t2[:])
                d2 = mk("d2"); nc.vector.tensor_mul(d2[:], t3[:], t4[:])
                det = mk("det"); nc.vector.tensor_sub(det[:], d1[:], d2[:])
                nd = mk("nd"); nc.vector.tensor_scalar_mul(nd[:], det[:], -1.0)
                adet = mk("adet"); nc.vector.tensor_max(adet[:], det[:], nd[:])
                am = mk("am")
                nc.vector.tensor_scalar_max(am[:], adet[:], 2.0 * AREA_EPS)
                rc = mk("rc"); nc.vector.reciprocal(rc[:], am[:])
                vm = mk("vm")
                nc.vector.tensor_single_scalar(vm[:], adet[:], 2.0 * AREA_EPS,
                                               op=mybir.AluOpType.is_gt)
                rcm = mk("rcm"); nc.vector.tensor_mul(rcm[:], rc[:], vm[:])
                # inv = kappa * valid / (4*area) = kappa * valid / (2*|det|)
                inv = mk("inv")
                nc.vector.tensor_scalar(inv[:], rcm[:], 0.5, kap_b[:],
                                        op0=mybir.AluOpType.mult,
                                        op1=mybir.AluOpType.mult)
                b0 = mk("b0"); nc.vector.tensor_sub(b0[:], t4[:], t2[:])
                b2 = mk("b2"); nc.vector.tensor_scalar_mul(b2[:], t4[:], -1.0)
                c0 = mk("c0"); nc.vector.tensor_sub(c0[:], t3[:], t1[:])
                c1 = mk("c1"); nc.vector.tensor_scalar_mul(c1[:], t3[:], -1.0)
                B = [b0, t2, b2]
                C = [c0, c1, t1]

                for p in range(3):
                    for q in range(p, 3):
                        m1 = mk("m1"); nc.vector.tensor_mul(m1[:], B[p][:], B[q][:])
                        m2 = mk("m2"); nc.vector.tensor_mul(m2[:], C[p][:], C[q][:])
                        kv = mk("kv"); nc.vector.tensor_add(kv[:], m1[:], m2[:])
                        kvi = mk("kvi"); nc.vector.tensor_mul(kvi[:], kv[:], inv[:])
                        for (pp, qq) in ([(p, q)] if p == q else [(p, q), (q, p)]):
                            dd = (int(offs[tau, qq, 0] - offs[tau, pp, 0]),
                                  int(offs[tau, qq, 1] - offs[tau, pp, 1]))
                            oa, ob = int(offs[tau, pp, 0]), int(offs[tau, pp, 1])
                            tgt = (W1[dd] if oa == 1 else W0[dd])[0:M, ob:ob + M]
                            nc.vector.tensor_add(tgt, tgt, kvi[:])

                fsum = mk("fsum"); nc.vector.tensor_add(fsum[:], fs[0], fs[1])
                nc.vector.tensor_add(fsum[:], fsum[:], fs[2])
                dv = mk("dv"); nc.vector.tensor_mul(dv[:], adet[:], vm[:])
                fe = mk("fe"); nc.vector.tensor_mul(fe[:], dv[:], fsum[:])
                nc.vector.tensor_scalar_mul(fe[:], fe[:], 1.0 / 18.0)
                for p in range(3):
                    oa, ob = int(offs[tau, p, 0]), int(offs[tau, p, 1])
                    tgt = (F1acc if oa == 1 else F0acc)[0:M, ob:ob + M]
                    nc.vector.tensor_add(tgt, tgt, fe[:])

            # realign the cell-row-1 accumulators down one partition and fold
            W = {}
            for d in dirs:
                W[d] = W0[d]
                if d in needs_shift:
                    sh = wk.tile([N, N], f32, tag="sh", name=f"sh_{d}")
                    nc.vector.memzero(sh[:])
                    nc.gpsimd.dma_start(sh[1:N, :], W1[d][0:M, :])
                    nc.vector.tensor_add(W[d][:], W[d][:], sh[:])
            Fsh = wk.tile([N, N], f32, tag="Fsh")
            nc.vector.memzero(Fsh[:])
            nc.gpsimd.dma_start(Fsh[1:N, :], F1acc[0:M, :])
            F = wk.tile([N, N], f32, tag="F")
            nc.vector.tensor_add(F[:], F0acc[:], Fsh[:])

            def matvec(dst, u):
                """dst = K @ u over the full node plane (stencil form).

                Row shifts of u are realigned via DMA; column shifts are AP
                offsets with the op restricted to the valid column range (the
                W planes are zero outside their support).
                """
                u_up = wk.tile([N, N], f32, tag="u_up")   # u_up[a] = u[a+1]
                u_dn = wk.tile([N, N], f32, tag="u_dn")   # u_dn[a] = u[a-1]
                nc.vector.memzero(u_up[:])
                nc.vector.memzero(u_dn[:])
                nc.gpsimd.dma_start(u_up[0:M, :], u[1:N, :])
                nc.gpsimd.dma_start(u_dn[1:N, :], u[0:M, :])
                shifted = {-1: u_dn, 0: u, 1: u_up}
                tmp = wk.tile([N, N], f32, tag="mvtmp")
                first = True
                for (da, db) in dirs:
                    uu = shifted[da]
                    if db >= 0:
                        osl = np.s_[:, 0:N - db]
                        isl = np.s_[:, db:N]
                    else:
                        osl = np.s_[:, -db:N]
                        isl = np.s_[:, 0:N + db]
                    if first:
                        nc.vector.memzero(dst[:])
                        nc.vector.tensor_mul(dst[osl], W[(da, db)][osl], uu[isl])
                        first = False
                    else:
                        nc.vector.tensor_mul(tmp[osl], W[(da, db)][osl], uu[isl])
                        nc.vector.tensor_add(dst[osl], dst[osl], tmp[osl])

            def dst_solve(zdst, r_plane):
                """zdst[NI,NI] (SBUF) = approx K_free^{-1} r_plane_interior."""
                rt = wk.tile([NI, NI], f32, tag="rt")
                nc.gpsimd.dma_start(rt[:], r_plane[1:N - 1, 1:N - 1])
                h_ps = ps.tile([NI, NI], f32, tag="mm")
                nc.tensor.matmul(h_ps[:], rt[:], S[:], start=True, stop=True)
                h = wk.tile([NI, NI], f32, tag="h")
                nc.vector.tensor_copy(h[:], h_ps[:])
                t_ps = ps.tile([NI, NI], f32, tag="mm")
                nc.tensor.matmul(t_ps[:], h[:], S[:], start=True, stop=True)
                t2s = wk.tile([NI, NI], f32, tag="t2s")
                nc.vector.tensor_mul(t2s[:], t_ps[:], ILK[:])
                p_ps = ps.tile([NI, NI], f32, tag="mm")
                nc.tensor.matmul(p_ps[:], t2s[:], S[:], start=True, stop=True)
                p1 = wk.tile([NI, NI], f32, tag="p1")
                nc.vector.tensor_copy(p1[:], p_ps[:])
                z_ps = ps.tile([NI, NI], f32, tag="mm")
                nc.tensor.matmul(z_ps[:], p1[:], S[:], start=True, stop=True)
                nc.vector.tensor_copy(zdst[:], z_ps[:])

            # F0 = F - K u_bc ; u = u_bc + pad(DST(F0_int))
            acc = wk.tile([N, N], f32, tag="acc")
            matvec(acc, UBC)
            F0 = wk.tile([N, N], f32, tag="F0")
            nc.vector.tensor_sub(F0[:], F[:], acc[:])
            z = wk.tile([NI, NI], f32, tag="z")
            dst_solve(z, F0)
            zp = wk.tile([N, N], f32, tag="zp")
            nc.vector.memzero(zp[:])
            nc.gpsimd.dma_start(zp[1:N - 1, 1:N - 1], z[:])
            u = wk.tile([N, N], f32, tag="u")
            nc.vector.tensor_add(u[:], UBC[:], zp[:])

            # one refinement sweep against the assembled K (u's boundary
            # carries u_bc, so K@u already includes the Dirichlet columns)
            acc2 = wk.tile([N, N], f32, tag="acc2")
            matvec(acc2, u)
            r1 = wk.tile([N, N], f32, tag="r1")
            nc.vector.tensor_sub(r1[:], F[:], acc2[:])
            z2 = wk.tile([NI, NI], f32, tag="z2")
            dst_solve(z2, r1)
            zp2 = wk.tile([N, N], f32, tag="zp2")
            nc.vector.memzero(zp2[:])
            nc.gpsimd.dma_start(zp2[1:N - 1, 1:N - 1], z2[:])
            nc.vector.tensor_add(u[:], u[:], zp2[:])

            nc.gpsimd.dma_start(d_U[:], u[:])

    nc.compile()
    return nc


def _prepare_maps(f, nodes, kappa, dir_vals):
    X = np.ascontiguousarray(nodes[:, 0].reshape(N, N).astype(np.float32))
    Y = np.ascontiguousarray(nodes[:, 1].reshape(N, N).astype(np.float32))
    FG = np.ascontiguousarray(f.reshape(N, N).astype(np.float32))
    UBC = np.zeros((N, N), np.float32)
    # dir_idx is validated (== boundary ids, sorted) in _host_plan; pure
    # permutation scatter of the input values, no arithmetic
    idx = np.arange(N * N).reshape(N, N)
    bmask = np.zeros(N * N, bool)
    bmask[idx[0, :]] = True; bmask[idx[-1, :]] = True
    bmask[idx[:, 0]] = True; bmask[idx[:, -1]] = True
    UBC.reshape(-1)[np.nonzero(bmask)[0]] = dir_vals.astype(np.float32)
    # algorithm constants: DST matrix and eigenvalue plane for the grid size
    k = np.arange(1, NI + 1)
    S = np.sin(np.pi * np.outer(k, k) / (NI + 1)).astype(np.float32)
    lam = 4.0 * np.sin(np.pi * k / (2 * (NI + 1))) ** 2
    IL = ((2.0 / (NI + 1)) ** 2 / (lam[:, None] + lam[None, :])).astype(np.float32)
    KAP = kappa.reshape(1, 1).astype(np.float32)
    m = {"X": X, "Y": Y, "FG": FG, "UBC": UBC, "S": S, "IL": IL, "KAP": KAP}
    return [dict(m) for _ in range(NCORES)]


def kernel(f, nodes, kappa, dir_vals, elements, free_idx, dir_idx,
           _want_trace=False):
    f = np.asarray(f); nodes = np.asarray(nodes); kappa = np.asarray(kappa)
    dir_vals = np.asarray(dir_vals); elements = np.asarray(elements)
    free_idx = np.asarray(free_idx); dir_idx = np.asarray(dir_idx)

    offs = _host_plan(elements, free_idx, dir_idx)
    key = offs.tobytes()
    if key not in _CACHE:
        _CACHE[key] = _build_program(offs)
    nc = _CACHE[key]

    in_maps = _prepare_maps(f, nodes, kappa, dir_vals)
    res = run_bass_kernel_spmd(nc, in_maps, list(range(NCORES)),
                               trace=_want_trace)
    u = res.results[0]["U"].reshape(-1).astype(np.float32)
    if _want_trace:
        kernel._last_result = res
    return u


# revision 11
# speedup vs baseline: 1.5215x; 1.5215x over previous
"""Differentiable FE solver (2D P1 FEM Poisson, 64x64 structured grid) on TRN2.

Pipeline (all floating-point work on device, replicated SPMD on 8 cores):
  1. Element assembly: per-element geometry (b, c, area), local stiffness
     Ke = kappa*(b_p b_q + c_p c_q)/(4 area) and load fe = area/3 * mean(f).
     The mesh topology (from the int32 `elements` input) is cell-regular, so
     every gather/scatter becomes a shifted 2D-slice add on 64x64 node planes
     -- no indexed DMA needed.  The assembled operator is kept in stencil form
     (7 direction planes side by side in one [64, 512] tile) instead of a
     dense 4096^2 K.
  2. Dirichlet elimination: F0 = F - K*u_bc (stencil matvec); boundary rows
     are dropped by the zero-padded transform matrices in step 3.
  3. Solve K_free u = F0 by DST-preconditioned iterative refinement: the exact
     inverse of the constant-coefficient Laplacian on the grid is
     S diag(1/(lam_i+lam_j)) S (S = 62x62 sine matrix), applied as 4 small PE
     matmuls.  Zero-padded variants of S (SP/SPR) fuse the interior
     extraction / padding into the transforms.  One refinement step against
     the *assembled* K (so the answer tracks the actual inputs, not the
     idealized operator) reaches ~1e-6 relative error.

Engine access patterns may only start at partitions 0/32/64/96, so all
partition-dimension (grid-row) shifts run as tiny PE matmuls against 0/1
shift matrices; free-dimension shifts are plain AP offsets.

Host side only derives integer layout plans from the int32 topology inputs,
reshapes/permutes arrays, and emits constant tables (sine matrices, shift
matrices, eigenvalue plane); every float computation happens in the kernel.
"""

import numpy as np

import concourse.bass as bass
import concourse.bacc as bacc
import concourse.mybir as mybir
import concourse.tile as tile
from concourse.bass_utils import run_bass_kernel_spmd

N = 64            # nodes per side
M = N - 1         # cells per side
NI = N - 2        # interior nodes per side
NCORES = 8
AREA_EPS = 1e-15

# stencil plane order: groups with equal row-shift (da) are contiguous and
# column-shift (db) ascends inside each group -- the batched matvec relies
# on both properties.  Index 7 is the load-vector plane F.
DIR_ORDER = [(-1, -1), (-1, 0), (0, -1), (0, 0), (0, 1), (1, 0), (1, 1)]
NPL = 8           # 7 stencil planes + F
VW = NPL * N      # 512: width of the plane-stack tiles

_CACHE = {}


def _host_plan(elements, free_idx, dir_idx):
    """Derive the cell-regular layout plan from int32 topology inputs."""
    el = elements.astype(np.int64)
    ga, gb = el // N, el % N
    ne = el.shape[0]
    assert ne == 2 * M * M, ne
    ncell = ne // 2
    ca, cb = np.meshgrid(np.arange(M), np.arange(M), indexing="ij")
    cells = np.stack([ca.ravel(), cb.ravel()], 1)
    offs = np.zeros((2, 3, 2), np.int64)
    for tau in (0, 1):
        es = slice(tau * ncell, (tau + 1) * ncell)
        for p in range(3):
            d = np.stack([ga[es, p], gb[es, p]], 1) - cells
            assert (d == d[0]).all(), "mesh is not cell-regular"
            assert d[0, 0] in (0, 1) and d[0, 1] in (0, 1)
            offs[tau, p] = d[0]
    for tau in (0, 1):
        for p in range(3):
            for q in range(3):
                d = (int(offs[tau, q, 0] - offs[tau, p, 0]),
                     int(offs[tau, q, 1] - offs[tau, p, 1]))
                assert d in DIR_ORDER, d
    idx = np.arange(N * N).reshape(N, N)
    bmask = np.zeros(N * N, bool)
    bmask[idx[0, :]] = True
    bmask[idx[-1, :]] = True
    bmask[idx[:, 0]] = True
    bmask[idx[:, -1]] = True
    assert (free_idx == np.nonzero(~bmask)[0]).all(), "free_idx mismatch"
    assert (dir_idx == np.nonzero(bmask)[0]).all(), "dir_idx mismatch"
    return offs


def _build_program(offs):
    f32 = mybir.dt.float32
    AT = mybir.AluOpType
    nc = bacc.Bacc("TRN2", target_bir_lowering=False, debug=False,
                   num_devices=NCORES)

    d_X = nc.dram_tensor("X", [N, N], f32, kind="ExternalInput")
    d_Y = nc.dram_tensor("Y", [N, N], f32, kind="ExternalInput")
    d_FG = nc.dram_tensor("FG", [N, N], f32, kind="ExternalInput")
    d_UBC = nc.dram_tensor("UBC", [N, N], f32, kind="ExternalInput")
    d_SP = nc.dram_tensor("SP", [N, NI], f32, kind="ExternalInput")
    d_SPR = nc.dram_tensor("SPR", [NI, N], f32, kind="ExternalInput")
    d_IL = nc.dram_tensor("IL", [NI, NI], f32, kind="ExternalInput")
    d_SHUD = nc.dram_tensor("SHUD", [N, 2 * N], f32, kind="ExternalInput")
    d_KAP = nc.dram_tensor("KAP", [1, 1], f32, kind="ExternalInput")
    d_U = nc.dram_tensor("U", [N, N], f32, kind="ExternalOutput")

    def ap(t, offset, pattern):
        base = t[:]
        return bass.AP(base.tensor, offset, [list(base.ap[0])] + pattern)

    with tile.TileContext(nc) as tc:
        with (
            tc.tile_pool(name="io", bufs=1) as io,
            tc.tile_pool(name="wk", bufs=1) as wk,
            tc.tile_pool(name="ps", bufs=1, space="PSUM") as ps,
        ):
            XYF = io.tile([N, 3 * N], f32, tag="XYF")
            UBC = io.tile([N, N], f32, tag="UBC")
            SP = io.tile([N, NI], f32, tag="SP")
            SPR = io.tile([NI, N], f32, tag="SPR")
            IL = io.tile([NI, NI], f32, tag="IL")
            SHUD = io.tile([N, 2 * N], f32, tag="SHUD")
            KAP = io.tile([1, 1], f32, tag="KAP")
            nc.gpsimd.dma_start(XYF[:, 0:N], d_X[:])
            nc.gpsimd.dma_start(XYF[:, N:2 * N], d_Y[:])
            nc.gpsimd.dma_start(XYF[:, 2 * N:3 * N], d_FG[:])
            nc.gpsimd.dma_start(UBC[:], d_UBC[:])
            nc.gpsimd.dma_start(SP[:], d_SP[:])
            nc.gpsimd.dma_start(SPR[:], d_SPR[:])
            nc.gpsimd.dma_start(IL[:], d_IL[:])
            nc.gpsimd.dma_start(SHUD[:], d_SHUD[:])
            nc.gpsimd.dma_start(KAP[:], d_KAP[:])

            # XYFS[a] = XYF[a+1]: row-shifted coordinate/load planes
            xyfs_ps = ps.tile([N, 3 * N], f32, tag="xyfs")
            nc.tensor.matmul(xyfs_ps[:], SHUD[:, 0:N], XYF[:],
                             start=True, stop=True)
            XYFS = wk.tile([N, 3 * N], f32, tag="XYFS")
            nc.vector.tensor_copy(XYFS[:], xyfs_ps[:])

            # broadcast kappa / (1/kappa) down the partition dim via the PE
            kinv = wk.tile([1, 1], f32, tag="kinv")
            nc.vector.reciprocal(kinv[:], KAP[:])
            ones = wk.tile([1, M], f32, tag="ones")
            nc.gpsimd.memset(ones[:], 1.0)
            kap_ps = ps.tile([M, 1], f32, tag="kbc")
            nc.tensor.matmul(kap_ps[:], ones[:], KAP[:], start=True, stop=True)
            kap_b = wk.tile([M, 1], f32, tag="kap_b")
            nc.vector.tensor_copy(kap_b[:], kap_ps[:])
            kinv_ps = ps.tile([M, 1], f32, tag="kbc")
            nc.tensor.matmul(kinv_ps[:], ones[:], kinv[:], start=True, stop=True)
            kinv_b = wk.tile([M, 1], f32, tag="kinv_b")
            nc.vector.tensor_copy(kinv_b[:], kinv_ps[:])
            ILK = wk.tile([NI, NI], f32, tag="ILK")
            nc.vector.tensor_scalar(ILK[:], IL[:], kinv_b[0:NI, 0:1], None,
                                    op0=AT.mult)

            # ---- element assembly, both triangle types batched ----
            # BC: 12 blocks of 64 cols (63 used): per tau [b0 b1 b2 c0 c1 c2]
            BC = wk.tile([M, 12 * N], f32, tag="BC")

            def vsrc(tau, p, comp):
                oa, ob = int(offs[tau, p, 0]), int(offs[tau, p, 1])
                t = XYFS if oa == 1 else XYF
                return t[0:M, comp * N + ob: comp * N + ob + M]

            for tau in (0, 1):
                base = tau * 6 * N
                cyc = [(1, 2), (2, 0), (0, 1)]  # b_p = y[p+1] - y[p+2] etc.
                for j, (a1, a2) in enumerate(cyc):
                    nc.vector.tensor_sub(BC[0:M, base + j * N: base + j * N + M],
                                         vsrc(tau, a1, 1), vsrc(tau, a2, 1))
                for j, (a1, a2) in enumerate(cyc):
                    nc.vector.tensor_sub(
                        BC[0:M, base + (3 + j) * N: base + (3 + j) * N + M],
                        vsrc(tau, a2, 0), vsrc(tau, a1, 0))

            def two_tau(t, blk):
                """AP over both tau halves of a 12-block tile: [M, 2, M]."""
                return ap(t, blk * N, [[6 * N, 2], [1, M]])

            def half2(t):
                """AP over a [M, 2*N] tile's two 64-col halves: [M, 2, M]."""
                return ap(t, 0, [[N, 2], [1, M]])

            def mk2(tag):
                return wk.tile([M, 2 * N], f32, tag=tag, name=tag)

            # det = c2*b1 - c1*b2  (both taus per op)
            d1 = mk2("d1"); nc.vector.tensor_mul(half2(d1), two_tau(BC, 5), two_tau(BC, 1))
            d2 = mk2("d2"); nc.vector.tensor_mul(half2(d2), two_tau(BC, 4), two_tau(BC, 2))
            det = mk2("det"); nc.vector.tensor_sub(half2(det), half2(d1), half2(d2))
            nd = mk2("nd"); nc.vector.tensor_scalar_mul(half2(nd), half2(det), -1.0)
            adet = mk2("adet"); nc.vector.tensor_max(half2(adet), half2(det), half2(nd))
            am = mk2("am"); nc.vector.tensor_scalar_max(half2(am), half2(adet), 2.0 * AREA_EPS)
            rc = mk2("rc"); nc.vector.reciprocal(half2(rc), half2(am))
            vm = mk2("vm")
            nc.vector.tensor_single_scalar(half2(vm), half2(adet), 2.0 * AREA_EPS,
                                           op=AT.is_gt)
            rcm = mk2("rcm"); nc.vector.tensor_mul(half2(rcm), half2(rc), half2(vm))
            # inv = kappa * valid / (4*area) = kappa * valid / (2*|det|)
            inv = mk2("inv")
            nc.vector.tensor_scalar(half2(inv), half2(rcm), 0.5, kap_b[:],
                                    op0=AT.mult, op1=AT.mult)

            # all 18 pair products (b_p b_q + c_p c_q) * inv, one block each
            KV = wk.tile([M, 18 * N], f32, tag="KV")
            KVC = wk.tile([M, 18 * N], f32, tag="KVC")
            kv_out = ap(KV, 0, [[9 * N, 2], [N, 9], [1, M]])
            kvc_out = ap(KVC, 0, [[9 * N, 2], [N, 9], [1, M]])
            for tau in (0, 1):  # ISA allows at most 3 free AP dims per op
                nc.vector.tensor_mul(
                    ap(KV, tau * 9 * N, [[N, 9], [1, M]]),
                    ap(BC, tau * 6 * N, [[N, 3], [0, 3], [1, M]]),
                    ap(BC, tau * 6 * N, [[0, 3], [N, 3], [1, M]]))
                nc.vector.tensor_mul(
                    ap(KVC, tau * 9 * N, [[N, 9], [1, M]]),
                    ap(BC, (tau * 6 + 3) * N, [[N, 3], [0, 3], [1, M]]),
                    ap(BC, (tau * 6 + 3) * N, [[0, 3], [N, 3], [1, M]]))
            nc.vector.tensor_add(kv_out, kv_out, kvc_out)
            inv_bc = ap(inv, 0, [[N, 2], [0, 9], [1, M]])
            nc.vector.tensor_mul(kv_out, kv_out, inv_bc)

            # load vector: fe = (|det|/18) * (f0+f1+f2) * valid
            fsum = mk2("fsum")
            for tau in (0, 1):
                h = fsum[0:M, tau * N: tau * N + M]
                nc.vector.tensor_add(h, vsrc(tau, 0, 2), vsrc(tau, 1, 2))
                nc.vector.tensor_add(h, h, vsrc(tau, 2, 2))
            dv = mk2("dv"); nc.vector.tensor_mul(half2(dv), half2(adet), half2(vm))
            fe = mk2("fe")
            nc.vector.scalar_tensor_tensor(half2(fe), half2(dv), 1.0 / 18.0,
                                           half2(fsum), op0=AT.mult, op1=AT.mult)

            # scatter-add into the plane stacks (V0: cell-row-aligned,
            # V1: contributions from cell-row-offset-1 vertices)
            V0 = wk.tile([N, VW], f32, tag="V0")
            V1 = wk.tile([N, VW], f32, tag="V1")
            nc.vector.memzero(V0[:])
            nc.vector.memzero(V1[:])
            for tau in (0, 1):
                for p in range(3):
                    oa, ob = int(offs[tau, p, 0]), int(offs[tau, p, 1])
                    V = V1 if oa == 1 else V0
                    for q in range(3):
                        d = (int(offs[tau, q, 0] - offs[tau, p, 0]),
                             int(offs[tau, q, 1] - offs[tau, p, 1]))
                        col = DIR_ORDER.index(d) * N + ob
                        src = KV[0:M, (tau * 9 + 3 * p + q) * N:
                                      (tau * 9 + 3 * p + q) * N + M]
                        tgt = V[0:M, col: col + M]
                        nc.vector.tensor_add(tgt, tgt, src)
                    ftgt = V[0:M, 7 * N + ob: 7 * N + ob + M]
                    nc.vector.tensor_add(ftgt, ftgt,
                                         fe[0:M, tau * N: tau * N + M])

            # fold: node row = cell row + 1 for V1 -> shift down one row
            v1_ps = ps.tile([N, VW], f32, tag="v1f")
            nc.tensor.matmul(v1_ps[:], SHUD[:, N:2 * N], V1[:],
                             start=True, stop=True)
            Vall = wk.tile([N, VW], f32, tag="Vall")
            nc.vector.tensor_add(Vall[:], V0[:], v1_ps[:])
            F_ap = Vall[:, 7 * N: 8 * N]

            # ---- stencil matvec: y = K @ u ----
            UM = wk.tile([N, 200], f32, tag="UM")   # [pad dn pad u up pad]
            nc.vector.memzero(UM[:])
            DN_B, U_B, UP_B = 1, 66, 130
            GRP = [(0, 2, DN_B - 1), (2, 3, U_B - 1), (5, 2, UP_B)]

            def matvec(dst, u, kvt, updn_ps):
                nc.tensor.matmul(updn_ps[:], SHUD[:], u, start=True, stop=True)
                nc.vector.tensor_copy(UM[:, U_B:U_B + N], u)
                nc.vector.tensor_copy(UM[:, UP_B:UP_B + N], updn_ps[0:N, :])
                nc.vector.tensor_copy(UM[:, DN_B:DN_B + N], updn_ps[N:2 * N, :])
                for (p0, cnt, ubase) in GRP:
                    nc.vector.tensor_mul(
                        ap(kvt, p0 * N, [[N, cnt], [1, N]]),
                        ap(Vall, p0 * N, [[N, cnt], [1, N]]),
                        ap(UM, ubase, [[1, cnt], [1, N]]))
                nc.vector.tensor_reduce(
                    dst, ap(kvt, 0, [[1, N], [N, 7]]),
                    axis=mybir.AxisListType.X, op=AT.add)

            def dst_solve(z_ps, r, h, hs, t2s, p1s):
                """z_ps [N,N] (PSUM) = padded K_free^{-1} r_interior."""
                nc.tensor.matmul(h[:], r, SP[:], start=True, stop=True)
                nc.vector.tensor_copy(hs[:], h[:])
                t_ps = ps.tile([NI, NI], f32, tag="mm", bufs=3)
                nc.tensor.matmul(t_ps[:], hs[:], SP[:], start=True, stop=True)
                nc.vector.tensor_mul(t2s[:], t_ps[:], ILK[:])
                p_ps = ps.tile([NI, N], f32, tag="mm", bufs=3)
                nc.tensor.matmul(p_ps[:], t2s[:], SPR[:], start=True, stop=True)
                nc.vector.tensor_copy(p1s[:], p_ps[:])
                nc.tensor.matmul(z_ps[:], p1s[:], SPR[:], start=True, stop=True)

            KVT = wk.tile([N, 7 * N], f32, tag="KVT")
            acc = wk.tile([N, N], f32, tag="acc")
            ud_ps = ps.tile([2 * N, N], f32, tag="updn")
            matvec(acc[:], UBC[:], KVT, ud_ps)
            r0 = wk.tile([N, N], f32, tag="r0")
            nc.vector.tensor_sub(r0[:], F_ap, acc[:])

            h1 = ps.tile([N, NI], f32, tag="mm", bufs=3)
            hs1 = wk.tile([N, NI], f32, tag="hs")
            t2s1 = wk.tile([NI, NI], f32, tag="t2s")
            p1s1 = wk.tile([NI, N], f32, tag="p1s")
            z1 = ps.tile([N, N], f32, tag="mm", bufs=3)
            dst_solve(z1, r0[:], h1, hs1, t2s1, p1s1)
            u = wk.tile([N, N], f32, tag="u")
            nc.vector.tensor_add(u[:], UBC[:], z1[:])

            # one refinement sweep against the assembled K (u's boundary
            # carries u_bc, so K@u already includes the Dirichlet columns)
            KVT2 = wk.tile([N, 7 * N], f32, tag="KVT2")
            acc2 = wk.tile([N, N], f32, tag="acc2")
            ud_ps2 = ps.tile([2 * N, N], f32, tag="updn")
            matvec(acc2[:], u[:], KVT2, ud_ps2)
            r1 = wk.tile([N, N], f32, tag="r1")
            nc.vector.tensor_sub(r1[:], F_ap, acc2[:])

            h2 = ps.tile([N, NI], f32, tag="mm", bufs=3)
            hs2 = wk.tile([N, NI], f32, tag="hs2")
            t2s2 = wk.tile([NI, NI], f32, tag="t2s2")
            p1s2 = wk.tile([NI, N], f32, tag="p1s2")
            z2 = ps.tile([N, N], f32, tag="mm", bufs=3)
            dst_solve(z2, r1[:], h2, hs2, t2s2, p1s2)
            u2 = wk.tile([N, N], f32, tag="u2")
            nc.vector.tensor_add(u2[:], u[:], z2[:])

            nc.gpsimd.dma_start(d_U[:], u2[:])

    nc.compile()
    return nc


def _prepare_maps(f, nodes, kappa, dir_vals):
    X = np.ascontiguousarray(nodes[:, 0].reshape(N, N).astype(np.float32))
    Y = np.ascontiguousarray(nodes[:, 1].reshape(N, N).astype(np.float32))
    FG = np.ascontiguousarray(f.reshape(N, N).astype(np.float32))
    UBC = np.zeros((N, N), np.float32)
    # dir_idx is validated (== boundary ids, sorted) in _host_plan; pure
    # permutation scatter of the input values, no arithmetic
    idx = np.arange(N * N).reshape(N, N)
    bmask = np.zeros(N * N, bool)
    bmask[idx[0, :]] = True; bmask[idx[-1, :]] = True
    bmask[idx[:, 0]] = True; bmask[idx[:, -1]] = True
    UBC.reshape(-1)[np.nonzero(bmask)[0]] = dir_vals.astype(np.float32)
    # algorithm constants: zero-padded DST matrices, eigenvalue plane,
    # row-shift matrices -- all derived from the grid size alone
    k = np.arange(1, NI + 1)
    S = np.sin(np.pi * np.outer(k, k) / (NI + 1)).astype(np.float32)
    SP = np.zeros((N, NI), np.float32)
    SP[1:N - 1, :] = S
    SPR = np.ascontiguousarray(SP.T)
    lam = 4.0 * np.sin(np.pi * k / (2 * (NI + 1))) ** 2
    IL = ((2.0 / (NI + 1)) ** 2 / (lam[:, None] + lam[None, :])).astype(np.float32)
    SHUD = np.zeros((N, 2 * N), np.float32)
    for m in range(N):
        if m + 1 < N:
            SHUD[m + 1, m] = 1.0          # up: out[m] = in[m+1]
        if m - 1 >= 0:
            SHUD[m - 1, N + m] = 1.0      # down: out[m] = in[m-1]
    KAP = kappa.reshape(1, 1).astype(np.float32)
    m = {"X": X, "Y": Y, "FG": FG, "UBC": UBC, "SP": SP, "SPR": SPR,
         "IL": IL, "SHUD": SHUD, "KAP": KAP}
    return [dict(m) for _ in range(NCORES)]


def kernel(f, nodes, kappa, dir_vals, elements, free_idx, dir_idx,
           _want_trace=False):
    f = np.asarray(f); nodes = np.asarray(nodes); kappa = np.asarray(kappa)
    dir_vals = np.asarray(dir_vals); elements = np.asarray(elements)
    free_idx = np.asarray(free_idx); dir_idx = np.asarray(dir_idx)

    offs = _host_plan(elements, free_idx, dir_idx)
    key = offs.tobytes()
    if key not in _CACHE:
        _CACHE[key] = _build_program(offs)
    nc = _CACHE[key]

    in_maps = _prepare_maps(f, nodes, kappa, dir_vals)
    res = run_bass_kernel_spmd(nc, in_maps, list(range(NCORES)),
                               trace=_want_trace)
    u = res.results[0]["U"].reshape(-1).astype(np.float32)
    if _want_trace:
        kernel._last_result = res
    return u


# revision 12
# speedup vs baseline: 1.6197x; 1.0645x over previous
"""Differentiable FE solver (2D P1 FEM Poisson, 64x64 structured grid) on TRN2.

Pipeline (all floating-point work on device, replicated SPMD on 8 cores):
  1. Element assembly: per-element geometry (b, c, area), local stiffness
     Ke = kappa*(b_p b_q + c_p c_q)/(4 area) and load fe = area/3 * mean(f).
     The mesh topology (from the int32 `elements` input) is cell-regular, so
     every gather/scatter becomes a shifted 2D-slice add on 64x64 node planes
     -- no indexed DMA needed.  The assembled operator is kept in stencil form
     (7 direction planes side by side in one [64, 512] tile) instead of a
     dense 4096^2 K.
  2. Dirichlet elimination: F0 = F - K*u_bc (stencil matvec); boundary rows
     are dropped by the zero-padded transform matrices in step 3.
  3. Solve K_free u = F0 by DST-preconditioned iterative refinement: the exact
     inverse of the constant-coefficient Laplacian on the grid is
     S diag(1/(lam_i+lam_j)) S (S = 62x62 sine matrix), applied as 4 small PE
     matmuls.  Zero-padded variants of S (SP/SPR) fuse the interior
     extraction / padding into the transforms.  One refinement step against
     the *assembled* K (so the answer tracks the actual inputs, not the
     idealized operator) reaches ~1e-6 relative error.

Engine access patterns may only start at partitions 0/32/64/96, so all
partition-dimension (grid-row) shifts run as tiny PE matmuls against 0/1
shift matrices; free-dimension shifts are plain AP offsets.

Host side only derives integer layout plans from the int32 topology inputs,
reshapes/permutes arrays, and emits constant tables (sine matrices, shift
matrices, eigenvalue plane); every float computation happens in the kernel.
"""

import numpy as np

import concourse.bass as bass
import concourse.bacc as bacc
import concourse.mybir as mybir
import concourse.tile as tile
from concourse.bass_utils import run_bass_kernel_spmd

N = 64            # nodes per side
M = N - 1         # cells per side
NI = N - 2        # interior nodes per side
NCORES = 8
AREA_EPS = 1e-15

# stencil plane order: groups with equal row-shift (da) are contiguous and
# column-shift (db) ascends inside each group -- the batched matvec relies
# on both properties.  Index 7 is the load-vector plane F.
DIR_ORDER = [(-1, -1), (-1, 0), (0, -1), (0, 0), (0, 1), (1, 0), (1, 1)]
NPL = 8           # 7 stencil planes + F
VW = NPL * N      # 512: width of the plane-stack tiles

_CACHE = {}


def _host_plan(elements, free_idx, dir_idx):
    """Derive the cell-regular layout plan from int32 topology inputs."""
    el = elements.astype(np.int64)
    ga, gb = el // N, el % N
    ne = el.shape[0]
    assert ne == 2 * M * M, ne
    ncell = ne // 2
    ca, cb = np.meshgrid(np.arange(M), np.arange(M), indexing="ij")
    cells = np.stack([ca.ravel(), cb.ravel()], 1)
    offs = np.zeros((2, 3, 2), np.int64)
    for tau in (0, 1):
        es = slice(tau * ncell, (tau + 1) * ncell)
        for p in range(3):
            d = np.stack([ga[es, p], gb[es, p]], 1) - cells
            assert (d == d[0]).all(), "mesh is not cell-regular"
            assert d[0, 0] in (0, 1) and d[0, 1] in (0, 1)
            offs[tau, p] = d[0]
    for tau in (0, 1):
        for p in range(3):
            for q in range(3):
                d = (int(offs[tau, q, 0] - offs[tau, p, 0]),
                     int(offs[tau, q, 1] - offs[tau, p, 1]))
                assert d in DIR_ORDER, d
    idx = np.arange(N * N).reshape(N, N)
    bmask = np.zeros(N * N, bool)
    bmask[idx[0, :]] = True
    bmask[idx[-1, :]] = True
    bmask[idx[:, 0]] = True
    bmask[idx[:, -1]] = True
    assert (free_idx == np.nonzero(~bmask)[0]).all(), "free_idx mismatch"
    assert (dir_idx == np.nonzero(bmask)[0]).all(), "dir_idx mismatch"
    return offs


def _build_program(offs):
    f32 = mybir.dt.float32
    AT = mybir.AluOpType
    nc = bacc.Bacc("TRN2", target_bir_lowering=False, debug=False,
                   num_devices=NCORES)

    d_X = nc.dram_tensor("X", [N, N], f32, kind="ExternalInput")
    d_Y = nc.dram_tensor("Y", [N, N], f32, kind="ExternalInput")
    d_FG = nc.dram_tensor("FG", [N, N], f32, kind="ExternalInput")
    d_UBC = nc.dram_tensor("UBC", [N, N], f32, kind="ExternalInput")
    d_SP = nc.dram_tensor("SP", [N, NI], f32, kind="ExternalInput")
    d_SPR = nc.dram_tensor("SPR", [NI, N], f32, kind="ExternalInput")
    d_IL = nc.dram_tensor("IL", [NI, NI], f32, kind="ExternalInput")
    d_SHUD = nc.dram_tensor("SHUD", [N, 2 * N], f32, kind="ExternalInput")
    d_KAP = nc.dram_tensor("KAP", [1, 1], f32, kind="ExternalInput")
    d_U = nc.dram_tensor("U", [N, N], f32, kind="ExternalOutput")

    def ap(t, offset, pattern):
        base = t[:]
        return bass.AP(base.tensor, offset, [list(base.ap[0])] + pattern)

    with tile.TileContext(nc) as tc:
        with (
            tc.tile_pool(name="io", bufs=1) as io,
            tc.tile_pool(name="wk", bufs=1) as wk,
            tc.tile_pool(name="ps", bufs=1, space="PSUM") as ps,
        ):
            XYF = io.tile([N, 3 * N], f32, tag="XYF")
            UBC = io.tile([N, N], f32, tag="UBC")
            SP = io.tile([N, NI], f32, tag="SP")
            SPR = io.tile([NI, N], f32, tag="SPR")
            IL = io.tile([NI, NI], f32, tag="IL")
            SHUD = io.tile([N, 2 * N], f32, tag="SHUD")
            KAP = io.tile([1, 1], f32, tag="KAP")
            nc.gpsimd.dma_start(SHUD[:], d_SHUD[:])
            nc.gpsimd.dma_start(XYF[:, 0:N], d_X[:])
            nc.gpsimd.dma_start(XYF[:, N:2 * N], d_Y[:])
            nc.gpsimd.dma_start(XYF[:, 2 * N:3 * N], d_FG[:])
            nc.gpsimd.dma_start(KAP[:], d_KAP[:])
            nc.scalar.dma_start(UBC[:], d_UBC[:])
            nc.scalar.dma_start(SP[:], d_SP[:])
            nc.scalar.dma_start(SPR[:], d_SPR[:])
            nc.scalar.dma_start(IL[:], d_IL[:])

            # XYFS[a] = XYF[a+1]: row-shifted coordinate/load planes
            xyfs_ps = ps.tile([N, 3 * N], f32, tag="xyfs")
            nc.tensor.matmul(xyfs_ps[:], SHUD[:, 0:N], XYF[:],
                             start=True, stop=True)
            XYFS = wk.tile([N, 3 * N], f32, tag="XYFS")
            nc.vector.tensor_copy(XYFS[:], xyfs_ps[:])

            # broadcast kappa / (1/kappa) down the partition dim via the PE
            kinv = wk.tile([1, 1], f32, tag="kinv")
            nc.vector.reciprocal(kinv[:], KAP[:])
            ones = wk.tile([1, M], f32, tag="ones")
            nc.gpsimd.memset(ones[:], 1.0)
            kap_ps = ps.tile([M, 1], f32, tag="kbc")
            nc.tensor.matmul(kap_ps[:], ones[:], KAP[:], start=True, stop=True)
            kap_b = wk.tile([M, 1], f32, tag="kap_b")
            nc.vector.tensor_copy(kap_b[:], kap_ps[:])
            kinv_ps = ps.tile([M, 1], f32, tag="kbc")
            nc.tensor.matmul(kinv_ps[:], ones[:], kinv[:], start=True, stop=True)
            kinv_b = wk.tile([M, 1], f32, tag="kinv_b")
            nc.vector.tensor_copy(kinv_b[:], kinv_ps[:])
            ILK = wk.tile([NI, NI], f32, tag="ILK")
            nc.vector.tensor_scalar(ILK[:], IL[:], kinv_b[0:NI, 0:1], None,
                                    op0=AT.mult)

            # ---- element assembly, both triangle types batched ----
            # BC: 12 blocks of 64 cols (63 used): per tau [b0 b1 b2 c0 c1 c2]
            BC = wk.tile([M, 12 * N], f32, tag="BC")

            def vsrc(tau, p, comp):
                oa, ob = int(offs[tau, p, 0]), int(offs[tau, p, 1])
                t = XYFS if oa == 1 else XYF
                return t[0:M, comp * N + ob: comp * N + ob + M]

            for tau in (0, 1):
                base = tau * 6 * N
                cyc = [(1, 2), (2, 0), (0, 1)]  # b_p = y[p+1] - y[p+2] etc.
                for j, (a1, a2) in enumerate(cyc):
                    nc.vector.tensor_sub(BC[0:M, base + j * N: base + j * N + M],
                                         vsrc(tau, a1, 1), vsrc(tau, a2, 1))
                for j, (a1, a2) in enumerate(cyc):
                    nc.vector.tensor_sub(
                        BC[0:M, base + (3 + j) * N: base + (3 + j) * N + M],
                        vsrc(tau, a2, 0), vsrc(tau, a1, 0))

            def two_tau(t, blk):
                """AP over both tau halves of a 12-block tile: [M, 2, M]."""
                return ap(t, blk * N, [[6 * N, 2], [1, M]])

            def half2(t):
                """AP over a [M, 2*N] tile's two 64-col halves: [M, 2, M]."""
                return ap(t, 0, [[N, 2], [1, M]])

            def mk2(tag):
                return wk.tile([M, 2 * N], f32, tag=tag, name=tag)

            # det = c2*b1 - c1*b2  (both taus per op)
            d1 = mk2("d1"); nc.vector.tensor_mul(half2(d1), two_tau(BC, 5), two_tau(BC, 1))
            d2 = mk2("d2"); nc.vector.tensor_mul(half2(d2), two_tau(BC, 4), two_tau(BC, 2))
            det = mk2("det"); nc.vector.tensor_sub(half2(det), half2(d1), half2(d2))
            nd = mk2("nd"); nc.vector.tensor_scalar_mul(half2(nd), half2(det), -1.0)
            adet = mk2("adet"); nc.vector.tensor_max(half2(adet), half2(det), half2(nd))
            am = mk2("am"); nc.vector.tensor_scalar_max(half2(am), half2(adet), 2.0 * AREA_EPS)
            rc = mk2("rc"); nc.vector.reciprocal(half2(rc), half2(am))
            vm = mk2("vm")
            nc.vector.tensor_single_scalar(half2(vm), half2(adet), 2.0 * AREA_EPS,
                                           op=AT.is_gt)
            rcm = mk2("rcm"); nc.vector.tensor_mul(half2(rcm), half2(rc), half2(vm))
            # inv = kappa * valid / (4*area) = kappa * valid / (2*|det|)
            inv = mk2("inv")
            nc.vector.tensor_scalar(half2(inv), half2(rcm), 0.5, kap_b[:],
                                    op0=AT.mult, op1=AT.mult)

            # all 18 pair products (b_p b_q + c_p c_q) * inv, one block each
            KV = wk.tile([M, 18 * N], f32, tag="KV")
            KVC = wk.tile([M, 18 * N], f32, tag="KVC")
            kv_out = ap(KV, 0, [[9 * N, 2], [N, 9], [1, M]])
            kvc_out = ap(KVC, 0, [[9 * N, 2], [N, 9], [1, M]])
            for tau in (0, 1):  # ISA allows at most 3 free AP dims per op
                nc.vector.tensor_mul(
                    ap(KV, tau * 9 * N, [[N, 9], [1, M]]),
                    ap(BC, tau * 6 * N, [[N, 3], [0, 3], [1, M]]),
                    ap(BC, tau * 6 * N, [[0, 3], [N, 3], [1, M]]))
                nc.vector.tensor_mul(
                    ap(KVC, tau * 9 * N, [[N, 9], [1, M]]),
                    ap(BC, (tau * 6 + 3) * N, [[N, 3], [0, 3], [1, M]]),
                    ap(BC, (tau * 6 + 3) * N, [[0, 3], [N, 3], [1, M]]))
            nc.vector.tensor_add(kv_out, kv_out, kvc_out)
            inv_bc = ap(inv, 0, [[N, 2], [0, 9], [1, M]])
            nc.vector.tensor_mul(kv_out, kv_out, inv_bc)

            # load vector: fe = (|det|/18) * (f0+f1+f2) * valid
            fsum = mk2("fsum")
            for tau in (0, 1):
                h = fsum[0:M, tau * N: tau * N + M]
                nc.vector.tensor_add(h, vsrc(tau, 0, 2), vsrc(tau, 1, 2))
                nc.vector.tensor_add(h, h, vsrc(tau, 2, 2))
            dv = mk2("dv"); nc.vector.tensor_mul(half2(dv), half2(adet), half2(vm))
            fe = mk2("fe")
            nc.vector.scalar_tensor_tensor(half2(fe), half2(dv), 1.0 / 18.0,
                                           half2(fsum), op0=AT.mult, op1=AT.mult)

            # scatter-add into the plane stacks (V0: cell-row-aligned,
            # V1: contributions from cell-row-offset-1 vertices)
            V0 = wk.tile([N, VW], f32, tag="V0")
            V1 = wk.tile([N, VW], f32, tag="V1")
            nc.gpsimd.memzero(V0[:])
            nc.vector.memzero(V1[:])
            for tau in (0, 1):
                for p in range(3):
                    oa, ob = int(offs[tau, p, 0]), int(offs[tau, p, 1])
                    V = V1 if oa == 1 else V0
                    eng = nc.vector if oa == 1 else nc.gpsimd
                    for q in range(3):
                        d = (int(offs[tau, q, 0] - offs[tau, p, 0]),
                             int(offs[tau, q, 1] - offs[tau, p, 1]))
                        col = DIR_ORDER.index(d) * N + ob
                        src = KV[0:M, (tau * 9 + 3 * p + q) * N:
                                      (tau * 9 + 3 * p + q) * N + M]
                        tgt = V[0:M, col: col + M]
                        eng.tensor_add(tgt, tgt, src)
                    ftgt = V[0:M, 7 * N + ob: 7 * N + ob + M]
                    eng.tensor_add(ftgt, ftgt,
                                   fe[0:M, tau * N: tau * N + M])

            # fold: node row = cell row + 1 for V1 -> shift down one row
            v1_ps = ps.tile([N, VW], f32, tag="v1f")
            nc.tensor.matmul(v1_ps[:], SHUD[:, N:2 * N], V1[:],
                             start=True, stop=True)
            Vall = wk.tile([N, VW], f32, tag="Vall")
            nc.vector.tensor_add(Vall[:], V0[:], v1_ps[:])
            F_ap = Vall[:, 7 * N: 8 * N]

            # ---- stencil matvec: y = K @ u ----
            UM = wk.tile([N, 200], f32, tag="UM")   # [pad dn pad u up pad]
            nc.gpsimd.memzero(UM[:])
            DN_B, U_B, UP_B = 1, 66, 130
            GRP = [(0, 2, DN_B - 1), (2, 3, U_B - 1), (5, 2, UP_B)]

            def matvec(dst, u, kvt, updn_ps):
                nc.tensor.matmul(updn_ps[:], SHUD[:], u, start=True, stop=True)
                nc.vector.tensor_copy(UM[:, U_B:U_B + N], u)
                nc.vector.tensor_copy(UM[:, UP_B:UP_B + N], updn_ps[0:N, :])
                nc.vector.tensor_copy(UM[:, DN_B:DN_B + N], updn_ps[N:2 * N, :])
                for (p0, cnt, ubase) in GRP:
                    nc.vector.tensor_mul(
                        ap(kvt, p0 * N, [[N, cnt], [1, N]]),
                        ap(Vall, p0 * N, [[N, cnt], [1, N]]),
                        ap(UM, ubase, [[1, cnt], [1, N]]))
                nc.vector.tensor_reduce(
                    dst, ap(kvt, 0, [[1, N], [N, 7]]),
                    axis=mybir.AxisListType.X, op=AT.add)

            def dst_solve(z_ps, r, h, hs, t2s, p1s):
                """z_ps [N,N] (PSUM) = padded K_free^{-1} r_interior."""
                nc.tensor.matmul(h[:], r, SP[:], start=True, stop=True)
                nc.vector.tensor_copy(hs[:], h[:])
                t_ps = ps.tile([NI, NI], f32, tag="mm", bufs=3)
                nc.tensor.matmul(t_ps[:], hs[:], SP[:], start=True, stop=True)
                nc.vector.tensor_mul(t2s[:], t_ps[:], ILK[:])
                p_ps = ps.tile([NI, N], f32, tag="mm", bufs=3)
                nc.tensor.matmul(p_ps[:], t2s[:], SPR[:], start=True, stop=True)
                nc.vector.tensor_copy(p1s[:], p_ps[:])
                nc.tensor.matmul(z_ps[:], p1s[:], SPR[:], start=True, stop=True)

            KVT = wk.tile([N, 7 * N], f32, tag="KVT")
            acc = wk.tile([N, N], f32, tag="acc")
            ud_ps = ps.tile([2 * N, N], f32, tag="updn")
            matvec(acc[:], UBC[:], KVT, ud_ps)
            r0 = wk.tile([N, N], f32, tag="r0")
            nc.vector.tensor_sub(r0[:], F_ap, acc[:])

            h1 = ps.tile([N, NI], f32, tag="mm", bufs=3)
            hs1 = wk.tile([N, NI], f32, tag="hs")
            t2s1 = wk.tile([NI, NI], f32, tag="t2s")
            p1s1 = wk.tile([NI, N], f32, tag="p1s")
            z1 = ps.tile([N, N], f32, tag="mm", bufs=3)
            dst_solve(z1, r0[:], h1, hs1, t2s1, p1s1)
            u = wk.tile([N, N], f32, tag="u")
            nc.vector.tensor_add(u[:], UBC[:], z1[:])

            # one refinement sweep against the assembled K (u's boundary
            # carries u_bc, so K@u already includes the Dirichlet columns)
            KVT2 = wk.tile([N, 7 * N], f32, tag="KVT2")
            acc2 = wk.tile([N, N], f32, tag="acc2")
            ud_ps2 = ps.tile([2 * N, N], f32, tag="updn")
            matvec(acc2[:], u[:], KVT2, ud_ps2)
            r1 = wk.tile([N, N], f32, tag="r1")
            nc.vector.tensor_sub(r1[:], F_ap, acc2[:])

            h2 = ps.tile([N, NI], f32, tag="mm", bufs=3)
            hs2 = wk.tile([N, NI], f32, tag="hs2")
            t2s2 = wk.tile([NI, NI], f32, tag="t2s2")
            p1s2 = wk.tile([NI, N], f32, tag="p1s2")
            z2 = ps.tile([N, N], f32, tag="mm", bufs=3)
            dst_solve(z2, r1[:], h2, hs2, t2s2, p1s2)
            u2 = wk.tile([N, N], f32, tag="u2")
            nc.vector.tensor_add(u2[:], u[:], z2[:])

            nc.gpsimd.dma_start(d_U[:], u2[:])

    nc.compile()
    return nc


def _prepare_maps(f, nodes, kappa, dir_vals):
    X = np.ascontiguousarray(nodes[:, 0].reshape(N, N).astype(np.float32))
    Y = np.ascontiguousarray(nodes[:, 1].reshape(N, N).astype(np.float32))
    FG = np.ascontiguousarray(f.reshape(N, N).astype(np.float32))
    UBC = np.zeros((N, N), np.float32)
    # dir_idx is validated (== boundary ids, sorted) in _host_plan; pure
    # permutation scatter of the input values, no arithmetic
    idx = np.arange(N * N).reshape(N, N)
    bmask = np.zeros(N * N, bool)
    bmask[idx[0, :]] = True; bmask[idx[-1, :]] = True
    bmask[idx[:, 0]] = True; bmask[idx[:, -1]] = True
    UBC.reshape(-1)[np.nonzero(bmask)[0]] = dir_vals.astype(np.float32)
    # algorithm constants: zero-padded DST matrices, eigenvalue plane,
    # row-shift matrices -- all derived from the grid size alone
    k = np.arange(1, NI + 1)
    S = np.sin(np.pi * np.outer(k, k) / (NI + 1)).astype(np.float32)
    SP = np.zeros((N, NI), np.float32)
    SP[1:N - 1, :] = S
    SPR = np.ascontiguousarray(SP.T)
    lam = 4.0 * np.sin(np.pi * k / (2 * (NI + 1))) ** 2
    IL = ((2.0 / (NI + 1)) ** 2 / (lam[:, None] + lam[None, :])).astype(np.float32)
    SHUD = np.zeros((N, 2 * N), np.float32)
    for m in range(N):
        if m + 1 < N:
            SHUD[m + 1, m] = 1.0          # up: out[m] = in[m+1]
        if m - 1 >= 0:
            SHUD[m - 1, N + m] = 1.0      # down: out[m] = in[m-1]
    KAP = kappa.reshape(1, 1).astype(np.float32)
    m = {"X": X, "Y": Y, "FG": FG, "UBC": UBC, "SP": SP, "SPR": SPR,
         "IL": IL, "SHUD": SHUD, "KAP": KAP}
    return [dict(m) for _ in range(NCORES)]


def kernel(f, nodes, kappa, dir_vals, elements, free_idx, dir_idx,
           _want_trace=False):
    f = np.asarray(f); nodes = np.asarray(nodes); kappa = np.asarray(kappa)
    dir_vals = np.asarray(dir_vals); elements = np.asarray(elements)
    free_idx = np.asarray(free_idx); dir_idx = np.asarray(dir_idx)

    offs = _host_plan(elements, free_idx, dir_idx)
    key = offs.tobytes()
    if key not in _CACHE:
        _CACHE[key] = _build_program(offs)
    nc = _CACHE[key]

    in_maps = _prepare_maps(f, nodes, kappa, dir_vals)
    res = run_bass_kernel_spmd(nc, in_maps, list(range(NCORES)),
                               trace=_want_trace)
    u = res.results[0]["U"].reshape(-1).astype(np.float32)
    if _want_trace:
        kernel._last_result = res
    return u


# revision 14
# speedup vs baseline: 1.6969x; 1.0477x over previous
"""Differentiable FE solver (2D P1 FEM Poisson, 64x64 structured grid) on TRN2.

Pipeline (all floating-point work on device, replicated SPMD on 8 cores):
  1. Element assembly: per-element geometry (b, c, area), local stiffness
     Ke = kappa*(b_p b_q + c_p c_q)/(4 area) and load fe = area/3 * mean(f).
     The mesh topology (from the int32 `elements` input) is cell-regular, so
     every gather/scatter becomes a shifted 2D-slice add on 64x64 node planes
     -- no indexed DMA needed.  The assembled operator is kept in stencil form
     (7 direction planes side by side in one [64, 512] tile) instead of a
     dense 4096^2 K.
  2. Dirichlet elimination: F0 = F - K*u_bc (stencil matvec); boundary rows
     are dropped by the zero-padded transform matrices in step 3.
  3. Solve K_free u = F0 by DST-preconditioned iterative refinement: the exact
     inverse of the constant-coefficient Laplacian on the grid is
     S diag(1/(lam_i+lam_j)) S (S = 62x62 sine matrix), applied as 4 small PE
     matmuls.  Zero-padded variants of S (SP/SPR) fuse the interior
     extraction / padding into the transforms.  One refinement step against
     the *assembled* K (so the answer tracks the actual inputs, not the
     idealized operator) reaches ~1e-6 relative error.

Engine access patterns may only start at partitions 0/32/64/96, so all
partition-dimension (grid-row) shifts run as tiny PE matmuls against 0/1
shift matrices; free-dimension shifts are plain AP offsets.

Host side only derives integer layout plans from the int32 topology inputs,
reshapes/permutes arrays, and emits constant tables (sine matrices, shift
matrices, eigenvalue plane); every float computation happens in the kernel.
"""

import numpy as np

import concourse.bass as bass
import concourse.bacc as bacc
import concourse.mybir as mybir
import concourse.tile as tile
from concourse.bass_utils import run_bass_kernel_spmd

N = 64            # nodes per side
M = N - 1         # cells per side
NI = N - 2        # interior nodes per side
NCORES = 8
AREA_EPS = 1e-15

# stencil plane order: groups with equal row-shift (da) are contiguous and
# column-shift (db) ascends inside each group -- the batched matvec relies
# on both properties.  Index 7 is the load-vector plane F.
DIR_ORDER = [(-1, -1), (-1, 0), (0, -1), (0, 0), (0, 1), (1, 0), (1, 1)]
NPL = 8           # 7 stencil planes + F
VW = NPL * N      # 512: width of the plane-stack tiles
# packed constant-block column layout (single DMA): SP | SPR | IL | SHUD |
# UBC-mega (pre-shifted u_bc planes, a pure host-side permutation) | kappa
SP_C, SPR_C, IL_C = 0, NI, NI + N
SHUD_C = NI + N + NI
UBCM_C = SHUD_C + 2 * N
KAP_C = UBCM_C + 196
CW = KAP_C + 1

_CACHE = {}


def _host_plan(elements, free_idx, dir_idx):
    """Derive the cell-regular layout plan from int32 topology inputs."""
    el = elements.astype(np.int64)
    ga, gb = el // N, el % N
    ne = el.shape[0]
    assert ne == 2 * M * M, ne
    ncell = ne // 2
    ca, cb = np.meshgrid(np.arange(M), np.arange(M), indexing="ij")
    cells = np.stack([ca.ravel(), cb.ravel()], 1)
    offs = np.zeros((2, 3, 2), np.int64)
    for tau in (0, 1):
        es = slice(tau * ncell, (tau + 1) * ncell)
        for p in range(3):
            d = np.stack([ga[es, p], gb[es, p]], 1) - cells
            assert (d == d[0]).all(), "mesh is not cell-regular"
            assert d[0, 0] in (0, 1) and d[0, 1] in (0, 1)
            offs[tau, p] = d[0]
    for tau in (0, 1):
        for p in range(3):
            for q in range(3):
                d = (int(offs[tau, q, 0] - offs[tau, p, 0]),
                     int(offs[tau, q, 1] - offs[tau, p, 1]))
                assert d in DIR_ORDER, d
    idx = np.arange(N * N).reshape(N, N)
    bmask = np.zeros(N * N, bool)
    bmask[idx[0, :]] = True
    bmask[idx[-1, :]] = True
    bmask[idx[:, 0]] = True
    bmask[idx[:, -1]] = True
    assert (free_idx == np.nonzero(~bmask)[0]).all(), "free_idx mismatch"
    assert (dir_idx == np.nonzero(bmask)[0]).all(), "dir_idx mismatch"
    return offs


def _build_program(offs):
    f32 = mybir.dt.float32
    AT = mybir.AluOpType
    nc = bacc.Bacc("TRN2", target_bir_lowering=False, debug=False,
                   num_devices=NCORES)

    d_XYF = nc.dram_tensor("XYF", [N, 3 * N], f32, kind="ExternalInput")
    d_C = nc.dram_tensor("CONSTS", [N, CW], f32, kind="ExternalInput")
    d_U = nc.dram_tensor("U", [N, N], f32, kind="ExternalOutput")

    def ap(t, offset, pattern):
        base = t[:]
        return bass.AP(base.tensor, offset, [list(base.ap[0])] + pattern)

    with tile.TileContext(nc) as tc:
        with (
            tc.tile_pool(name="io", bufs=1) as io,
            tc.tile_pool(name="wk", bufs=1) as wk,
            tc.tile_pool(name="ps", bufs=1, space="PSUM") as ps,
        ):
            XYF = io.tile([N, 3 * N], f32, tag="XYF")
            C = io.tile([N, CW], f32, tag="CONSTS")
            nc.gpsimd.dma_start(C[:], d_C[:])
            nc.gpsimd.dma_start(XYF[:], d_XYF[:])
            SP = C[:, SP_C:SP_C + NI]
            SPR = C[0:NI, SPR_C:SPR_C + N]
            IL = C[0:NI, IL_C:IL_C + NI]
            SHUD = C[:, SHUD_C:SHUD_C + 2 * N]
            UBCM = C[:, UBCM_C:UBCM_C + 196]
            UBC = C[:, UBCM_C + 66:UBCM_C + 66 + N]
            KAP = C[0:1, KAP_C:KAP_C + 1]

            # XYFS[a] = XYF[a+1]: row-shifted coordinate/load planes
            xyfs_ps = ps.tile([N, 3 * N], f32, tag="xyfs")
            nc.tensor.matmul(xyfs_ps[:], C[:, SHUD_C:SHUD_C + N], XYF[:],
                             start=True, stop=True)
            XYFS = wk.tile([N, 3 * N], f32, tag="XYFS")
            nc.vector.tensor_copy(XYFS[:], xyfs_ps[:])

            # broadcast kappa / (1/kappa) down the partition dim via the PE
            kinv = wk.tile([1, 1], f32, tag="kinv")
            nc.vector.reciprocal(kinv[:], KAP)
            ones = wk.tile([1, M], f32, tag="ones")
            nc.gpsimd.memset(ones[:], 1.0)
            kap_ps = ps.tile([M, 1], f32, tag="kbc")
            nc.tensor.matmul(kap_ps[:], ones[:], KAP, start=True, stop=True)
            kap_b = wk.tile([M, 1], f32, tag="kap_b")
            nc.vector.tensor_copy(kap_b[:], kap_ps[:])
            kinv_ps = ps.tile([M, 1], f32, tag="kbc")
            nc.tensor.matmul(kinv_ps[:], ones[:], kinv[:], start=True, stop=True)
            kinv_b = wk.tile([M, 1], f32, tag="kinv_b")
            nc.vector.tensor_copy(kinv_b[:], kinv_ps[:])
            ILK = wk.tile([NI, NI], f32, tag="ILK")
            nc.vector.tensor_scalar(ILK[:], IL, kinv_b[0:NI, 0:1], None,
                                    op0=AT.mult)

            # ---- element assembly, both triangle types batched ----
            # BC: 12 blocks of 64 cols (63 used): per tau [b0 b1 b2 c0 c1 c2]
            BC = wk.tile([M, 12 * N], f32, tag="BC")

            def vsrc(tau, p, comp):
                oa, ob = int(offs[tau, p, 0]), int(offs[tau, p, 1])
                t = XYFS if oa == 1 else XYF
                return t[0:M, comp * N + ob: comp * N + ob + M]

            for tau in (0, 1):
                base = tau * 6 * N
                cyc = [(1, 2), (2, 0), (0, 1)]  # b_p = y[p+1] - y[p+2] etc.
                for j, (a1, a2) in enumerate(cyc):
                    nc.vector.tensor_sub(BC[0:M, base + j * N: base + j * N + M],
                                         vsrc(tau, a1, 1), vsrc(tau, a2, 1))
                for j, (a1, a2) in enumerate(cyc):
                    nc.vector.tensor_sub(
                        BC[0:M, base + (3 + j) * N: base + (3 + j) * N + M],
                        vsrc(tau, a2, 0), vsrc(tau, a1, 0))

            def two_tau(t, blk):
                """AP over both tau halves of a 12-block tile: [M, 2, M]."""
                return ap(t, blk * N, [[6 * N, 2], [1, M]])

            def half2(t):
                """AP over a [M, 2*N] tile's two 64-col halves: [M, 2, M]."""
                return ap(t, 0, [[N, 2], [1, M]])

            def mk2(tag):
                return wk.tile([M, 2 * N], f32, tag=tag, name=tag)

            # det = c2*b1 - c1*b2  (both taus per op)
            d1 = mk2("d1"); nc.vector.tensor_mul(half2(d1), two_tau(BC, 5), two_tau(BC, 1))
            d2 = mk2("d2"); nc.vector.tensor_mul(half2(d2), two_tau(BC, 4), two_tau(BC, 2))
            det = mk2("det"); nc.vector.tensor_sub(half2(det), half2(d1), half2(d2))
            nd = mk2("nd"); nc.vector.tensor_scalar_mul(half2(nd), half2(det), -1.0)
            adet = mk2("adet"); nc.vector.tensor_max(half2(adet), half2(det), half2(nd))
            am = mk2("am"); nc.vector.tensor_scalar_max(half2(am), half2(adet), 2.0 * AREA_EPS)
            rc = mk2("rc"); nc.vector.reciprocal(half2(rc), half2(am))
            vm = mk2("vm")
            nc.vector.tensor_single_scalar(half2(vm), half2(adet), 2.0 * AREA_EPS,
                                           op=AT.is_gt)
            rcm = mk2("rcm"); nc.vector.tensor_mul(half2(rcm), half2(rc), half2(vm))
            # inv = kappa * valid / (4*area) = kappa * valid / (2*|det|)
            inv = mk2("inv")
            nc.vector.tensor_scalar(half2(inv), half2(rcm), 0.5, kap_b[:],
                                    op0=AT.mult, op1=AT.mult)

            # all 18 pair products (b_p b_q + c_p c_q) * inv, one block each
            KV = wk.tile([M, 18 * N], f32, tag="KV")
            KVC = wk.tile([M, 18 * N], f32, tag="KVC")
            kv_out = ap(KV, 0, [[9 * N, 2], [N, 9], [1, M]])
            kvc_out = ap(KVC, 0, [[9 * N, 2], [N, 9], [1, M]])
            for tau in (0, 1):  # ISA allows at most 3 free AP dims per op
                nc.vector.tensor_mul(
                    ap(KV, tau * 9 * N, [[N, 9], [1, M]]),
                    ap(BC, tau * 6 * N, [[N, 3], [0, 3], [1, M]]),
                    ap(BC, tau * 6 * N, [[0, 3], [N, 3], [1, M]]))
                nc.vector.tensor_mul(
                    ap(KVC, tau * 9 * N, [[N, 9], [1, M]]),
                    ap(BC, (tau * 6 + 3) * N, [[N, 3], [0, 3], [1, M]]),
                    ap(BC, (tau * 6 + 3) * N, [[0, 3], [N, 3], [1, M]]))
            nc.vector.tensor_add(kv_out, kv_out, kvc_out)
            inv_bc = ap(inv, 0, [[N, 2], [0, 9], [1, M]])
            nc.vector.tensor_mul(kv_out, kv_out, inv_bc)

            # load vector: fe = (|det|/18) * (f0+f1+f2) * valid
            fsum = mk2("fsum")
            for tau in (0, 1):
                h = fsum[0:M, tau * N: tau * N + M]
                nc.vector.tensor_add(h, vsrc(tau, 0, 2), vsrc(tau, 1, 2))
                nc.vector.tensor_add(h, h, vsrc(tau, 2, 2))
            dv = mk2("dv"); nc.vector.tensor_mul(half2(dv), half2(adet), half2(vm))
            fe = mk2("fe")
            nc.vector.scalar_tensor_tensor(half2(fe), half2(dv), 1.0 / 18.0,
                                           half2(fsum), op0=AT.mult, op1=AT.mult)

            # scatter-add into the plane stacks (V0: cell-row-aligned,
            # V1: contributions from cell-row-offset-1 vertices)
            V0 = wk.tile([N, VW], f32, tag="V0")
            V1 = wk.tile([N, VW], f32, tag="V1")
            nc.gpsimd.memzero(V0[:])
            nc.vector.memzero(V1[:])
            for tau in (0, 1):
                for p in range(3):
                    oa, ob = int(offs[tau, p, 0]), int(offs[tau, p, 1])
                    V = V1 if oa == 1 else V0
                    eng = nc.vector
                    for q in range(3):
                        d = (int(offs[tau, q, 0] - offs[tau, p, 0]),
                             int(offs[tau, q, 1] - offs[tau, p, 1]))
                        col = DIR_ORDER.index(d) * N + ob
                        src = KV[0:M, (tau * 9 + 3 * p + q) * N:
                                      (tau * 9 + 3 * p + q) * N + M]
                        tgt = V[0:M, col: col + M]
                        eng.tensor_add(tgt, tgt, src)
                    ftgt = V[0:M, 7 * N + ob: 7 * N + ob + M]
                    eng.tensor_add(ftgt, ftgt,
                                   fe[0:M, tau * N: tau * N + M])

            # fold: node row = cell row + 1 for V1 -> shift down one row
            v1_ps = ps.tile([N, VW], f32, tag="v1f")
            nc.tensor.matmul(v1_ps[:], C[:, SHUD_C + N:SHUD_C + 2 * N], V1[:],
                             start=True, stop=True)
            Vall = wk.tile([N, VW], f32, tag="Vall")
            nc.vector.tensor_add(Vall[:], V0[:], v1_ps[:])
            F_ap = Vall[:, 7 * N: 8 * N]

            # ---- stencil matvec: y = K @ u ----
            UM = wk.tile([N, 200], f32, tag="UM")   # [pad dn pad u up pad]
            nc.gpsimd.memzero(UM[:])
            DN_B, U_B, UP_B = 1, 66, 130
            GRP = [(0, 2, DN_B - 1), (2, 3, U_B - 1), (5, 2, UP_B)]

            def matvec(dst, u, kvt, updn_ps, um_src=None):
                if um_src is None:
                    nc.tensor.matmul(updn_ps[:], SHUD, u, start=True, stop=True)
                    nc.vector.tensor_copy(UM[:, U_B:U_B + N], u)
                    nc.vector.tensor_copy(UM[:, UP_B:UP_B + N], updn_ps[0:N, :])
                    nc.vector.tensor_copy(UM[:, DN_B:DN_B + N], updn_ps[N:2 * N, :])
                    um_t, um_base = UM, 0
                else:
                    um_t, um_base = um_src
                for (p0, cnt, ubase) in GRP:
                    nc.vector.tensor_mul(
                        ap(kvt, p0 * N, [[N, cnt], [1, N]]),
                        ap(Vall, p0 * N, [[N, cnt], [1, N]]),
                        ap(um_t, um_base + ubase, [[1, cnt], [1, N]]))
                nc.vector.tensor_reduce(
                    dst, ap(kvt, 0, [[1, N], [N, 7]]),
                    axis=mybir.AxisListType.X, op=AT.add)

            def dst_solve(z_ps, r, h, hs, t2s, p1s):
                """z_ps [N,N] (PSUM) = padded K_free^{-1} r_interior."""
                nc.tensor.matmul(h[:], r, SP, start=True, stop=True)
                nc.vector.tensor_copy(hs[:], h[:])
                t_ps = ps.tile([NI, NI], f32, tag="mm", bufs=3)
                nc.tensor.matmul(t_ps[:], hs[:], SP, start=True, stop=True)
                nc.vector.tensor_mul(t2s[:], t_ps[:], ILK[:])
                p_ps = ps.tile([NI, N], f32, tag="mm", bufs=3)
                nc.tensor.matmul(p_ps[:], t2s[:], SPR, start=True, stop=True)
                nc.vector.tensor_copy(p1s[:], p_ps[:])
                nc.tensor.matmul(z_ps[:], p1s[:], SPR, start=True, stop=True)

            KVT = wk.tile([N, 7 * N], f32, tag="KVT")
            acc = wk.tile([N, N], f32, tag="acc")
            ud_ps = ps.tile([2 * N, N], f32, tag="updn")
            matvec(acc[:], None, KVT, ud_ps, um_src=(C, UBCM_C))
            r0 = wk.tile([N, N], f32, tag="r0")
            nc.vector.tensor_sub(r0[:], F_ap, acc[:])

            h1 = ps.tile([N, NI], f32, tag="mm", bufs=3)
            hs1 = wk.tile([N, NI], f32, tag="hs")
            t2s1 = wk.tile([NI, NI], f32, tag="t2s")
            p1s1 = wk.tile([NI, N], f32, tag="p1s")
            z1 = ps.tile([N, N], f32, tag="mm", bufs=3)
            dst_solve(z1, r0[:], h1, hs1, t2s1, p1s1)
            u = wk.tile([N, N], f32, tag="u")
            nc.vector.tensor_add(u[:], UBC, z1[:])

            # one refinement sweep against the assembled K (u's boundary
            # carries u_bc, so K@u already includes the Dirichlet columns)
            KVT2 = wk.tile([N, 7 * N], f32, tag="KVT2")
            acc2 = wk.tile([N, N], f32, tag="acc2")
            ud_ps2 = ps.tile([2 * N, N], f32, tag="updn")
            matvec(acc2[:], u[:], KVT2, ud_ps2)
            r1 = wk.tile([N, N], f32, tag="r1")
            nc.vector.tensor_sub(r1[:], F_ap, acc2[:])

            h2 = ps.tile([N, NI], f32, tag="mm", bufs=3)
            hs2 = wk.tile([N, NI], f32, tag="hs2")
            t2s2 = wk.tile([NI, NI], f32, tag="t2s2")
            p1s2 = wk.tile([NI, N], f32, tag="p1s2")
            z2 = ps.tile([N, N], f32, tag="mm", bufs=3)
            dst_solve(z2, r1[:], h2, hs2, t2s2, p1s2)
            u2 = wk.tile([N, N], f32, tag="u2")
            nc.vector.tensor_add(u2[:], u[:], z2[:])

            nc.gpsimd.dma_start(d_U[:], u2[:])

    nc.compile()
    return nc


def _prepare_maps(f, nodes, kappa, dir_vals):
    X = nodes[:, 0].reshape(N, N).astype(np.float32)
    Y = nodes[:, 1].reshape(N, N).astype(np.float32)
    FG = f.reshape(N, N).astype(np.float32)
    XYF = np.ascontiguousarray(np.concatenate([X, Y, FG], axis=1))
    UBC = np.zeros((N, N), np.float32)
    # dir_idx is validated (== boundary ids, sorted) in _host_plan; pure
    # permutation scatter of the input values, no arithmetic
    idx = np.arange(N * N).reshape(N, N)
    bmask = np.zeros(N * N, bool)
    bmask[idx[0, :]] = True; bmask[idx[-1, :]] = True
    bmask[idx[:, 0]] = True; bmask[idx[:, -1]] = True
    UBC.reshape(-1)[np.nonzero(bmask)[0]] = dir_vals.astype(np.float32)
    # algorithm constants: zero-padded DST matrices, eigenvalue plane,
    # row-shift matrices -- all derived from the grid size alone
    k = np.arange(1, NI + 1)
    S = np.sin(np.pi * np.outer(k, k) / (NI + 1)).astype(np.float32)
    C = np.zeros((N, CW), np.float32)
    C[1:N - 1, SP_C:SP_C + NI] = S
    C[0:NI, SPR_C + 1:SPR_C + 1 + NI] = S
    lam = 4.0 * np.sin(np.pi * k / (2 * (NI + 1))) ** 2
    C[0:NI, IL_C:IL_C + NI] = ((2.0 / (NI + 1)) ** 2
                               / (lam[:, None] + lam[None, :])).astype(np.float32)
    for m in range(N):
        if m + 1 < N:
            C[m + 1, SHUD_C + m] = 1.0          # up: out[m] = in[m+1]
        if m - 1 >= 0:
            C[m - 1, SHUD_C + N + m] = 1.0      # down: out[m] = in[m-1]
    # u_bc mega-plane: [pad | dn | pad | u | up | pad] row-shifted copies
    # (pure data movement of the already-scattered boundary values)
    C[:, UBCM_C + 66:UBCM_C + 130] = UBC
    C[0:N - 1, UBCM_C + 130:UBCM_C + 194] = UBC[1:N]
    C[1:N, UBCM_C + 1:UBCM_C + 65] = UBC[0:N - 1]
    C[0, KAP_C] = kappa.reshape(-1)[0]
    m = {"XYF": XYF, "CONSTS": C}
    return [dict(m) for _ in range(NCORES)]


def kernel(f, nodes, kappa, dir_vals, elements, free_idx, dir_idx,
           _want_trace=False):
    f = np.asarray(f); nodes = np.asarray(nodes); kappa = np.asarray(kappa)
    dir_vals = np.asarray(dir_vals); elements = np.asarray(elements)
    free_idx = np.asarray(free_idx); dir_idx = np.asarray(dir_idx)

    offs = _host_plan(elements, free_idx, dir_idx)
    key = offs.tobytes()
    if key not in _CACHE:
        _CACHE[key] = _build_program(offs)
    nc = _CACHE[key]

    in_maps = _prepare_maps(f, nodes, kappa, dir_vals)
    res = run_bass_kernel_spmd(nc, in_maps, list(range(NCORES)),
                               trace=_want_trace)
    u = res.results[0]["U"].reshape(-1).astype(np.float32)
    if _want_trace:
        kernel._last_result = res
    return u
